# revision 3
# baseline (speedup 1.0000x reference)
"""Trainium2 Bass kernel for the 10-class supervised-contrastive loss.

Problem shapes (hardcoded): preds [10, 2048, 128] f32, target [2048] int64,
log_vars [10] f32 -> scalar f32.

Sharding (8 cores, SPMD, identical program per core):
  - core c owns class c fully (upper-triangle strips 0..15 of the [B,B]
    exp-cosine matrix, exploiting symmetry)
  - cores 0-3 additionally own a quarter of class 8, cores 4-7 a quarter of
    class 9 (4 full-width 128-row strips, fed ROTATED via np.roll so every
    core statically computes local strips 0..3; rotation keeps the diagonal
    on the diagonal and row sums are permutation-equivariant).

Device computes ONLY the O(B^2) part, software-pipelined with lag 2 so PE
never waits on ACT: per unit C = G_a^T @ G_cols (bf16 matmul, f32 PSUM),
E = exp(C/T) -> sc bf16 (ACT), diag window overwritten with 1.0 on GPSIMD,
row sums of E on DVE (tensor_reduce), and for the symmetric half the column
sums of strictly-upper tiles accumulated into one [1, 2048] PSUM row (PE
matmuls with a ones-vector stationary -> no per-tile weight reloads), drained
to SBUF in 512-chunks as soon as each chunk's last contributor finishes.

Host does all O(B*D)/O(B*C) work: row-normalization, positive/all cosine
sums P/R, assembling Z from the device rowsum columns + csum row, masked
mean-log-prob and the uncertainty-weighted sum.
"""

import ml_dtypes
import numpy as np

import concourse.bacc as bacc
import concourse.bass as bass
import concourse.mybir as mybir
import concourse.tile as tile
from concourse.bass_utils import run_bass_kernel_spmd

NUM_CLASSES = 10
B = 2048
D = 128
T = 0.07
BASE_T = 0.07
N_CORES = 8

f32 = mybir.dt.float32
bf16 = mybir.dt.bfloat16
np_bf16 = ml_dtypes.bfloat16

# Slot-0 (own class, upper triangle) work units.  Each unit owns one
# [128, <=1024] PSUM tile / one ACT call; members are (strip, c0, c1, scoff):
# strip a covers absolute cols [c0, c1) placed at tile offset scoff.  Narrow
# strips are packed pairwise so their ACT spans merge.
UNITS0 = []
for _a in range(8):
    UNITS0.append([(_a, _a * 128, 1024, 0)])
    UNITS0.append([(_a, 1024, 2048, 0)])
UNITS0 += [
    [(8, 1024, 2048, 0)],
    [(9, 1152, 2048, 0)],
    [(10, 1280, 2048, 0), (14, 1792, 2048, 768)],
    [(11, 1408, 2048, 0), (13, 1664, 2048, 640)],
    [(12, 1536, 2048, 0), (15, 1920, 2048, 512)],
]
# Slot-1 (shared class, rotated): 4 full-width strips, two 1024-halves each.
UNITS1 = [(t, c0, c1) for t in range(4) for (c0, c1) in ((0, 1024), (1024, 2048))]

N_RS = sum(len(u) for u in UNITS0) + len(UNITS1)  # 24 + 8 = 32 rowsum cols

# csum 512-chunk q is complete once all its windows' last contributors ran:
# emit the drain copy right after the D-phase of these slot-0 unit indices.
CSUM_DRAIN_AFTER = {4: 0, 12: 1, 18: 2, 20: 3}

TRACE = False
LAST_RESULT = None


def _chunks512(c0, c1):
    """Consecutive <=512-wide pieces from c0 (keeps PSUM writes in-bank)."""
    out = []
    c = c0
    while c < c1:
        nxt = min(c1, c + 512)
        out.append((c, nxt))
        c = nxt
    return out


def _build_nc():
    nc = bacc.Bacc(None, target_bir_lowering=False)

    g_dram = [
        nc.dram_tensor(f"g{s}", [128, B], bf16, kind="ExternalInput")
        for s in range(2)
    ]
    ones_dram = nc.dram_tensor("ones1", [128, 1], bf16, kind="ExternalInput")
    rs_dram = nc.dram_tensor("rs", [128, N_RS], f32, kind="ExternalOutput")
    csum_dram = nc.dram_tensor("csum", [1, B], f32, kind="ExternalOutput")

    add = mybir.AluOpType.add
    AX = mybir.AxisListType.X
    EXP = mybir.ActivationFunctionType.Exp

    with tile.TileContext(nc) as tc:
        with (
            tc.tile_pool(name="const", bufs=1) as constp,
            tc.tile_pool(name="gmat", bufs=1) as gmatp,
            tc.tile_pool(name="scp", bufs=4) as scp,
        ):
            ones_sb = constp.tile([128, 1], bf16, tag="ones1")
            nc.sync.dma_start(ones_sb[:], ones_dram[:])
            # Warm the ACT exp table while input DMAs run.
            warm = constp.tile([128, 1], f32, tag="warm")
            nc.scalar.activation(warm[:], ones_sb[:], EXP, scale=1.0)

            # g0 in four 512-col pieces so strip 0's matmuls start early.
            g0 = gmatp.tile([128, B], bf16, tag="G0", name="G0")
            for q in range(4):
                nc.sync.dma_start(
                    g0[:, q * 512 : (q + 1) * 512],
                    g_dram[0][:, q * 512 : (q + 1) * 512],
                )
            g1 = gmatp.tile([128, B], bf16, tag="G1", name="G1")
            nc.sync.dma_start(g1[:], g_dram[1][:])
            G = [g0, g1]
            rs_sb = constp.tile([128, N_RS], f32, tag="rs")
            csum_sb = constp.tile([1, B], f32, tag="csum_sb")

            with (
                tc.tile_pool(name="cpp", bufs=2, space="PSUM") as cpp,
                tc.tile_pool(name="csp", bufs=1, space="PSUM") as csp,
            ):
                csum_ps = csp.tile([1, B], f32, tag="csum", bufs=1, name="csum")

                # Build the flat unit list: (slot, members) where members are
                # (strip, c0, c1, scoff).
                units = [(0, u) for u in UNITS0] + [
                    (1, [(t, c0, c1, 0)]) for (t, c0, c1) in UNITS1
                ]
                n_units = len(units)
                sc_tiles = [None] * n_units
                rs_col0 = [0] * n_units
                col = 0
                for k, (s, mem) in enumerate(units):
                    rs_col0[k] = col
                    col += len(mem)

                def phase_AB(k):
                    """PE mains + ACT exp + GPSIMD diag overwrite."""
                    s, mem = units[k]
                    cp = cpp.tile([128, 1024], f32, tag="cp")
                    sc = scp.tile([128, 1024], bf16, tag="sc")
                    sc_tiles[k] = sc
                    wtot = mem[-1][3] + (mem[-1][2] - mem[-1][1])
                    for a, c0, c1, off in mem:
                        lhsT = G[s][:, bass.ts(a, 128)]
                        for a0, a1 in _chunks512(c0, c1):
                            nc.tensor.matmul(
                                cp[:, off + a0 - c0 : off + a1 - c0],
                                lhsT,
                                G[s][:, a0:a1],
                                start=True,
                                stop=True,
                            )
                    nc.scalar.activation(
                        sc[:, 0:wtot], cp[:, 0:wtot], EXP, scale=1.0 / T
                    )
                    for a, c0, c1, off in mem:
                        if c0 <= a * 128 < c1:
                            # exp(diag)=huge but finite; overwrite with 1.0
                            # (same as masking pre-exp; host subtracts 1).
                            w0 = off + a * 128 - c0
                            nc.gpsimd.memset(sc[:, w0 : w0 + 128], 1.0)

                def phase_DE(k):
                    """PE csum matmuls + DVE rowsum reduces (+ csum drain)."""
                    s, mem = units[k]
                    sc = sc_tiles[k]
                    c = rs_col0[k]
                    for a, c0, c1, off in mem:
                        if s == 0:
                            lo = max(c0, (a + 1) * 128)
                            for wb in range(lo // 128, c1 // 128):
                                so = off + wb * 128 - c0
                                nc.tensor.matmul(
                                    csum_ps[0:1, wb * 128 : (wb + 1) * 128],
                                    ones_sb[:],
                                    sc[:, so : so + 128],
                                    start=(a == 0),
                                    stop=(a == wb - 1),
                                    skip_group_check=True,
                                )
                        nc.vector.tensor_reduce(
                            rs_sb[:, c : c + 1],
                            sc[:, off : off + (c1 - c0)],
                            axis=AX,
                            op=add,
                        )
                        c += 1
                    if s == 0 and k in CSUM_DRAIN_AFTER:
                        q = CSUM_DRAIN_AFTER[k]
                        nc.vector.tensor_copy(
                            csum_sb[0:1, q * 512 : (q + 1) * 512],
                            csum_ps[0:1, q * 512 : (q + 1) * 512],
                        )
                        if q == 3:
                            nc.sync.dma_start(csum_dram[:], csum_sb[:])

                for k in range(n_units):
                    phase_AB(k)
                    if k >= 2:
                        phase_DE(k - 2)
                phase_DE(n_units - 2)
                phase_DE(n_units - 1)

            nc.sync.dma_start(rs_dram[:], rs_sb[:])
    nc.finalize()
    return nc


_NC_CACHE = None


def _get_nc():
    global _NC_CACHE
    if _NC_CACHE is None:
        _NC_CACHE = _build_nc()
    return _NC_CACHE


def kernel(preds, target, log_vars):
    global LAST_RESULT
    preds = np.asarray(preds, dtype=np.float32)
    target = np.asarray(target)
    log_vars = np.asarray(log_vars, dtype=np.float32)

    onehot = (target[None, :] == np.arange(NUM_CLASSES, dtype=target.dtype)[:, None])
    onehot = onehot.astype(np.float64)  # [10, B]
    npos = onehot.sum(axis=1)  # [10]

    # Host prep: row-normalize (f32), cast bf16, feature-major layout.
    norms = np.sqrt((preds**2).sum(axis=2, dtype=np.float32))
    ghat = preds / norms[:, :, None]  # [10, B, D] f32
    gbf = ghat.astype(np_bf16)

    ones1 = np.ones((128, 1), dtype=np_bf16)

    in_maps = []
    for c in range(N_CORES):
        cls1 = 8 + c // 4
        off = 512 * (c % 4)
        g1 = np.roll(gbf[cls1], -off, axis=0) if off else gbf[cls1]
        in_maps.append(
            {
                "g0": np.ascontiguousarray(gbf[c].T),
                "g1": np.ascontiguousarray(g1.T),
                "ones1": ones1,
            }
        )

    nc = _get_nc()
    res = run_bass_kernel_spmd(nc, in_maps, list(range(N_CORES)), trace=TRACE)
    LAST_RESULT = res

    # Assemble Z (row sums of exp(cos/T), diag excluded) in f64.
    Z = np.zeros((NUM_CLASSES, B), dtype=np.float64)
    r128 = np.arange(128)
    for c in range(N_CORES):
        o = np.asarray(res.results[c]["rs"], dtype=np.float64)  # [128, N_RS]
        cs = np.asarray(res.results[c]["csum"], dtype=np.float64)  # [1, B]
        col = 0
        for unit in UNITS0:
            for a, c0, c1, off in unit:
                Z[c, a * 128 : (a + 1) * 128] += o[:, col]
                col += 1
        Z[c, 128:] += cs[0, 128:]
        cls1 = 8 + c // 4
        roff = 512 * (c % 4)
        for t in range(4):
            rows = (roff + t * 128 + r128) % B
            Z[cls1, rows] += o[:, col] + o[:, col + 1]
            col += 2
    Z -= 1.0  # remove diag exp(0)=1 contribution

    # Host-side O(B*D): positive/all cosine sums per class.
    g64 = ghat.astype(np.float64)
    P = np.empty((NUM_CLASSES, B), dtype=np.float64)
    R = np.empty((NUM_CLASSES, B), dtype=np.float64)
    for cls in range(NUM_CLASSES):
        g = g64[cls]
        P[cls] = g @ (g.T @ onehot[cls])
        R[cls] = g @ g.sum(axis=0)

    lab = onehot
    masked_cos = lab * P + (1.0 - lab) * (R - P)
    masked_logits_sum = (masked_cos - 1.0) / T
    cnt = lab * npos[:, None] + (1.0 - lab) * (B - npos[:, None]) - 1.0
    mlpp = masked_logits_sum / cnt - np.log(Z)
    losses = -(T / BASE_T) * mlpp.mean(axis=1)  # [10]
    lv = log_vars.astype(np.float64)
    final = np.sum(np.exp(-lv) * losses + lv)
    return np.float32(final)


# revision 4
# speedup vs baseline: 1.1264x; 1.1264x over previous
"""Trainium2 Bass kernel for the 10-class supervised-contrastive loss.

Problem shapes (hardcoded): preds [10, 2048, 128] f32, target [2048] int64,
log_vars [10] f32 -> scalar f32.

Sharding (8 cores, SPMD, identical program per core):
  - slot 0: core c owns class c fully (upper-triangle strips 0..15 of the
    [B,B] exp-cosine matrix, exploiting symmetry).
  - slot 1: cores 0-3 share class 8, cores 4-7 class 9.  Each core computes
    local strips {0,1,8,9} with tile-lengths {9,9,8,8} of its np.roll-rotated
    copy (rotations 0,2,4,6 tile units = 0,256,512,768 rows/cols).  This is
    an EXACT COVER: across the 4 rotations every unordered 128x128 tile pair
    of the class appears exactly once, so slot 1 also gets the 2x symmetry
    saving.  Row sums of skipped mirror tiles are recovered from column sums
    (csum rows), combined on the host in original coordinates.

Device computes ONLY the O(B^2) part, software-pipelined with lag 2: per
unit C = G_a^T @ G_cols (bf16 matmul, f32 PSUM), E = exp(C/T) -> sc bf16
(ACT), row sums on DVE (slot 0, diag overwritten to 1.0 on GPSIMD after the
exp) or via ACT accum_out (slot 1, diag masked on DVE before the exp), and
column sums of strictly-upper tiles accumulated into a [1, 2048] PSUM row
(PE matmuls with a ones-vector stationary, batched in <=512 pieces), drained
to SBUF as soon as each 512-chunk's last contributor finishes.  Slot 1's
csum reuses the same PSUM banks after slot 0's csum is drained.

Host does all O(B*D)/O(B*C) work: row-normalization, positive/all cosine
sums P/R, assembling Z from the device rowsum columns + csum rows, masked
mean-log-prob and the uncertainty-weighted sum.
"""

import ml_dtypes
import numpy as np

import concourse.bacc as bacc
import concourse.bass as bass
import concourse.mybir as mybir
import concourse.tile as tile
from concourse.bass_utils import run_bass_kernel_spmd

NUM_CLASSES = 10
B = 2048
D = 128
T = 0.07
BASE_T = 0.07
N_CORES = 8

f32 = mybir.dt.float32
bf16 = mybir.dt.bfloat16
np_bf16 = ml_dtypes.bfloat16

# Slot-0 (own class, upper triangle) units; members are (strip, c0, c1, off):
# strip a covers absolute cols [c0, c1) at PSUM-tile offset off.  Narrow
# strips are packed pairwise so their ACT spans merge.
UNITS0 = []
for _a in range(8):
    UNITS0.append([(_a, _a * 128, 1024, 0)])
    UNITS0.append([(_a, 1024, 2048, 0)])
UNITS0 += [
    [(8, 1024, 2048, 0)],
    [(9, 1152, 2048, 0)],
    [(10, 1280, 2048, 0), (14, 1792, 2048, 768)],
    [(11, 1408, 2048, 0), (13, 1664, 2048, 640)],
    [(12, 1536, 2048, 0), (15, 1920, 2048, 512)],
]

# Slot-1 units (local coords of the rotated class): strips {0,1,8,9} with
# tile ranges [a, a+9) / [a, a+8); strip 9 wraps (tile 16 -> window 0).
# Order A,B,D,C,E; A/B/D/C rowsum via ACT accum_out (single strip each),
# E via DVE reduces.
UNITS1 = [
    [(0, 0, 1024, 0)],                        # A: strip 0 main
    [(1, 128, 1152, 0)],                      # B: strip 1 main
    [(9, 1152, 2048, 0), (9, 0, 128, 896)],   # D: strip 9 main + wrap
    [(8, 1024, 2048, 0)],                     # C: strip 8
    [(0, 1024, 1152, 0), (1, 1152, 1280, 128)],  # E: leftovers s0/s1
]
S1_ACCUM = [True, True, True, True, False]

# csum1 matmul pieces per slot-1 unit: (c0, c1, member_idx, start, stop).
CSUM1_PIECES = [
    [(128, 256, 0, True, True), (256, 512, 0, True, False),
     (512, 1024, 0, True, False)],
    [(256, 512, 0, False, True), (512, 1024, 0, False, True),
     (1024, 1152, 0, True, False)],
    [(1280, 1536, 0, True, False), (1536, 2048, 0, True, False),
     (0, 128, 1, True, True)],
    [(1152, 1280, 0, True, False), (1280, 1536, 0, False, True),
     (1536, 2048, 0, False, True)],
    [(1024, 1152, 0, False, True), (1152, 1280, 1, False, True)],
]

N_RS = sum(len(u) for u in UNITS0) + 6  # 24 slot-0 + 4 accum + 2 reduce

# Slot-0 csum 512-chunk q is complete after these slot-0 units' D phases.
CSUM_DRAIN_AFTER = {4: 0, 12: 1, 18: 2, 20: 3}

TRACE = False
LAST_RESULT = None


def _csum0_pieces(a, c0, c1):
    """Slot-0 csum pieces for strip a's member [c0,c1): strictly-upper
    windows, first window alone when strip a is its last contributor
    (exact stop flag), rest split at absolute 512 boundaries."""
    lo = max(c0, (a + 1) * 128)
    if lo >= c1:
        return []
    pieces = []
    p = lo
    if lo == (a + 1) * 128:
        pieces.append((lo, lo + 128, a == 0, True))
        p = lo + 128
    while p < c1:
        nxt = min(c1, (p // 512 + 1) * 512)
        pieces.append((p, nxt, a == 0, False))
        p = nxt
    return pieces


def _build_nc():
    nc = bacc.Bacc(None, target_bir_lowering=False)

    g_dram = [
        nc.dram_tensor(f"g{s}", [128, B], bf16, kind="ExternalInput")
        for s in range(2)
    ]
    masknd_dram = nc.dram_tensor("masknd", [128, 128], f32, kind="ExternalInput")
    ones_dram = nc.dram_tensor("ones1", [128, 1], bf16, kind="ExternalInput")
    rs_dram = nc.dram_tensor("rs", [128, N_RS], f32, kind="ExternalOutput")
    csum_dram = nc.dram_tensor("csum", [1, B], f32, kind="ExternalOutput")
    csum1_dram = nc.dram_tensor("csum1", [1, B], f32, kind="ExternalOutput")

    add = mybir.AluOpType.add
    AX = mybir.AxisListType.X
    EXP = mybir.ActivationFunctionType.Exp

    with tile.TileContext(nc) as tc:
        with (
            tc.tile_pool(name="const", bufs=1) as constp,
            tc.tile_pool(name="gmat", bufs=1) as gmatp,
            tc.tile_pool(name="scp", bufs=4) as scp,
        ):
            # Warm the ACT exp table immediately (source via gpsimd memset,
            # no DMA dependency) so the ~2.7us table load hides under DMAs.
            warmsrc = constp.tile([128, 1], f32, tag="warmsrc")
            nc.gpsimd.memset(warmsrc[:], 0.0)
            warm = constp.tile([128, 1], f32, tag="warm")
            nc.scalar.activation(warm[:], warmsrc[:], EXP, scale=1.0)

            # Inputs on both HWDGE queues (sync + scalar) for parallelism.
            g0 = gmatp.tile([128, B], bf16, tag="G0", name="G0")
            for q in range(4):
                eng = nc.sync if q % 2 == 0 else nc.scalar
                eng.dma_start(
                    g0[:, q * 512 : (q + 1) * 512],
                    g_dram[0][:, q * 512 : (q + 1) * 512],
                )
            ones_sb = constp.tile([128, 1], bf16, tag="ones1")
            nc.sync.dma_start(ones_sb[:], ones_dram[:])
            g1 = gmatp.tile([128, B], bf16, tag="G1", name="G1")
            for q in range(4):
                eng = nc.sync if q % 2 == 0 else nc.scalar
                eng.dma_start(
                    g1[:, q * 512 : (q + 1) * 512],
                    g_dram[1][:, q * 512 : (q + 1) * 512],
                )
            masknd_sb = constp.tile([128, 128], f32, tag="masknd")
            nc.scalar.dma_start(masknd_sb[:], masknd_dram[:])
            G = [g0, g1]
            rs_sb = constp.tile([128, N_RS], f32, tag="rs")
            csum_sb = constp.tile([1, B], f32, tag="csum_sb")
            csum1_sb = constp.tile([1, B], f32, tag="csum1_sb")

            with (
                tc.tile_pool(name="cpp", bufs=2, space="PSUM") as cpp,
                tc.tile_pool(name="csp", bufs=1, space="PSUM") as csp,
            ):
                csum_ps = csp.tile([1, B], f32, tag="csum", bufs=1, name="csum")
                csum1_ps = [None]  # allocated after slot-0 csum drains

                units = [(0, u, None) for u in UNITS0] + [
                    (1, u, S1_ACCUM[i]) for i, u in enumerate(UNITS1)
                ]
                n_units = len(units)
                sc_tiles = [None] * n_units
                rs_col0 = [0] * n_units
                col = 0
                for k, (s, mem, acc) in enumerate(units):
                    rs_col0[k] = col
                    col += 1 if (s == 1 and acc) else len(mem)

                def phase_AB(k):
                    """PE mains; [DVE diag mask]; ACT exp (+accum);
                    [GPSIMD diag overwrite]."""
                    s, mem, acc = units[k]
                    cp = cpp.tile([128, 1024], f32, tag="cp")
                    sc = scp.tile([128, 1024], bf16, tag="sc")
                    sc_tiles[k] = sc
                    wtot = mem[-1][3] + (mem[-1][2] - mem[-1][1])
                    for a, c0, c1, off in mem:
                        lhsT = G[s][:, bass.ts(a, 128)]
                        p = c0
                        while p < c1:
                            nxt = min(c1, p + 512)
                            nc.tensor.matmul(
                                cp[:, off + p - c0 : off + nxt - c0],
                                lhsT,
                                G[s][:, p:nxt],
                                start=True,
                                stop=True,
                            )
                            p = nxt
                    if s == 1 and acc:
                        # Diag lands at cp[:,0:128] for all slot-1 mains;
                        # mask pre-exp so accum_out counts exp(0)=1.
                        for a, c0, c1, off in mem:
                            if c0 == a * 128:
                                nc.vector.tensor_mul(
                                    cp[:, off : off + 128],
                                    cp[:, off : off + 128],
                                    masknd_sb[:],
                                )
                        nc.scalar.activation(
                            sc[:, 0:wtot],
                            cp[:, 0:wtot],
                            EXP,
                            scale=1.0 / T,
                            accum_out=rs_sb[:, rs_col0[k] : rs_col0[k] + 1],
                        )
                    else:
                        nc.scalar.activation(
                            sc[:, 0:wtot], cp[:, 0:wtot], EXP, scale=1.0 / T
                        )
                        for a, c0, c1, off in mem:
                            if c0 <= a * 128 < c1:
                                w0 = off + a * 128 - c0
                                nc.gpsimd.memset(sc[:, w0 : w0 + 128], 1.0)

                def phase_DE(k):
                    """PE csum matmuls + DVE rowsum reduces + csum drains."""
                    s, mem, acc = units[k]
                    sc = sc_tiles[k]
                    c = rs_col0[k]
                    if s == 0:
                        for a, c0, c1, off in mem:
                            for p0, p1, st, sp in _csum0_pieces(a, c0, c1):
                                nc.tensor.matmul(
                                    csum_ps[0:1, p0:p1],
                                    ones_sb[:],
                                    sc[:, off + p0 - c0 : off + p1 - c0],
                                    start=st,
                                    stop=sp,
                                    skip_group_check=True,
                                )
                            nc.vector.tensor_reduce(
                                rs_sb[:, c : c + 1],
                                sc[:, off : off + (c1 - c0)],
                                axis=AX,
                                op=add,
                            )
                            c += 1
                        if k in CSUM_DRAIN_AFTER:
                            q = CSUM_DRAIN_AFTER[k]
                            nc.vector.tensor_copy(
                                csum_sb[0:1, q * 512 : (q + 1) * 512],
                                csum_ps[0:1, q * 512 : (q + 1) * 512],
                            )
                            if q == 3:
                                nc.sync.dma_start(csum_dram[:], csum_sb[:])
                    else:
                        k1 = k - len(UNITS0)
                        if csum1_ps[0] is None:
                            csum1_ps[0] = csp.tile(
                                [1, B], f32, tag="csum", bufs=1, name="csum1"
                            )
                        c1ps = csum1_ps[0]
                        for p0, p1, mi, st, sp in CSUM1_PIECES[k1]:
                            a, c0, cc1, off = mem[mi]
                            nc.tensor.matmul(
                                c1ps[0:1, p0:p1],
                                ones_sb[:],
                                sc[:, off + p0 - c0 : off + p1 - c0],
                                start=st,
                                stop=sp,
                                skip_group_check=True,
                            )
                        if not acc:
                            for a, c0, cc1, off in mem:
                                nc.vector.tensor_reduce(
                                    rs_sb[:, c : c + 1],
                                    sc[:, off : off + (cc1 - c0)],
                                    axis=AX,
                                    op=add,
                                )
                                c += 1
                        if k1 == 2:  # after D: windows 0..7 complete
                            nc.vector.tensor_copy(
                                csum1_sb[0:1, 0:1024], c1ps[0:1, 0:1024]
                            )
                        if k1 == 4:  # after E: windows 8..15 complete
                            nc.vector.tensor_copy(
                                csum1_sb[0:1, 1024:2048], c1ps[0:1, 1024:2048]
                            )
                            nc.sync.dma_start(csum1_dram[:], csum1_sb[:])

                for k in range(n_units):
                    phase_AB(k)
                    if k >= 2:
                        phase_DE(k - 2)
                phase_DE(n_units - 2)
                phase_DE(n_units - 1)

            nc.sync.dma_start(rs_dram[:], rs_sb[:])
    nc.finalize()
    return nc


_NC_CACHE = None


def _get_nc():
    global _NC_CACHE
    if _NC_CACHE is None:
        _NC_CACHE = _build_nc()
    return _NC_CACHE


def kernel(preds, target, log_vars):
    global LAST_RESULT
    preds = np.asarray(preds, dtype=np.float32)
    target = np.asarray(target)
    log_vars = np.asarray(log_vars, dtype=np.float32)

    onehot = (target[None, :] == np.arange(NUM_CLASSES, dtype=target.dtype)[:, None])
    onehot = onehot.astype(np.float64)  # [10, B]
    npos = onehot.sum(axis=1)  # [10]

    # Host prep: row-normalize (f32), cast bf16, feature-major layout.
    norms = np.sqrt((preds**2).sum(axis=2, dtype=np.float32))
    ghat = preds / norms[:, :, None]  # [10, B, D] f32
    gbf = ghat.astype(np_bf16)

    masknd = np.ascontiguousarray(1.0 - np.eye(128, dtype=np.float32))
    ones1 = np.ones((128, 1), dtype=np_bf16)

    in_maps = []
    for c in range(N_CORES):
        cls1 = 8 + c // 4
        off = 256 * (c % 4)
        g1 = np.roll(gbf[cls1], -off, axis=0) if off else gbf[cls1]
        in_maps.append(
            {
                "g0": np.ascontiguousarray(gbf[c].T),
                "g1": np.ascontiguousarray(g1.T),
                "masknd": masknd,
                "ones1": ones1,
            }
        )

    nc = _get_nc()
    res = run_bass_kernel_spmd(nc, in_maps, list(range(N_CORES)), trace=TRACE)
    LAST_RESULT = res

    # Assemble Z (row sums of exp(cos/T), diag excluded) in f64.
    Z = np.zeros((NUM_CLASSES, B), dtype=np.float64)
    r128 = np.arange(128)
    for c in range(N_CORES):
        o = np.asarray(res.results[c]["rs"], dtype=np.float64)  # [128, N_RS]
        cs = np.asarray(res.results[c]["csum"], dtype=np.float64)[0]  # [B]
        cs1 = np.asarray(res.results[c]["csum1"], dtype=np.float64)[0]  # [B]
        col = 0
        for unit in UNITS0:
            for a, c0, c1, off in unit:
                Z[c, a * 128 : (a + 1) * 128] += o[:, col]
                col += 1
        Z[c, 128:] += cs[128:]
        cls1 = 8 + c // 4
        roff = 256 * (c % 4)
        # rs cols 24..29: A(s0) B(s1) D(s9) C(s8) E0(s0 tail) E1(s1 tail)
        strip_sums = {
            0: o[:, 24] + o[:, 28],
            1: o[:, 25] + o[:, 29],
            9: o[:, 26],
            8: o[:, 27],
        }
        for a, v in strip_sums.items():
            rows = (a * 128 + r128 + roff) % B
            Z[cls1, rows] += v
        Z[cls1, (np.arange(B) + roff) % B] += cs1
    Z -= 1.0  # remove diag exp(0)=1 contribution

    # Host-side O(B*D): positive/all cosine sums per class.
    g64 = ghat.astype(np.float64)
    P = np.empty((NUM_CLASSES, B), dtype=np.float64)
    R = np.empty((NUM_CLASSES, B), dtype=np.float64)
    for cls in range(NUM_CLASSES):
        g = g64[cls]
        P[cls] = g @ (g.T @ onehot[cls])
        R[cls] = g @ g.sum(axis=0)

    lab = onehot
    masked_cos = lab * P + (1.0 - lab) * (R - P)
    masked_logits_sum = (masked_cos - 1.0) / T
    cnt = lab * npos[:, None] + (1.0 - lab) * (B - npos[:, None]) - 1.0
    mlpp = masked_logits_sum / cnt - np.log(Z)
    losses = -(T / BASE_T) * mlpp.mean(axis=1)  # [10]
    lv = log_vars.astype(np.float64)
    final = np.sum(np.exp(-lv) * losses + lv)
    return np.float32(final)
